# revision 1
# baseline (speedup 1.0000x reference)
"""AbstractBlast v5: bf16 merged-block hybrid.

x/Vt/y/z/S/U in bf16 (PSUM accumulation fp32), out fp32.
Stage-2 z_o = sum_j S[o,j,:]*y_j split across engines by o-block:
  - DVE o's: fused scalar_tensor_tensor MACs (bf16 2x mode, FD=1024)
  - PE  o's: plain 128x128 diag(S[o,j]) matmuls (bf16 FWL weight loads),
    diag tiles built on DVE from an identity via per-partition tensor_scalar.
Measured component costs (HW): DVE STT ~0.69us/pair, PE pair ~0.5-0.8us.
End-to-end rel err vs fp32 reference ~6e-3 (numpy-validated).
"""

import ml_dtypes
import numpy as np

import concourse.bass as bass
import concourse.mybir as mybir
from concourse.bass_utils import run_bass_kernel_spmd
from concourse.masks import make_identity
from concourse.tile import TileContext

F32 = mybir.dt.float32
BF16 = mybir.dt.bfloat16

B, T, D = 8, 1024, 4096
BIN, BOUT, BSIN, BSOUT, RANK = 16, 16, 256, 256, 128
NBLK = 2
NTOK = T // NBLK
# o-blocks whose stage-2 runs on PE (rest on DVE)
PE_OS = (0, 1, 2, 4, 5, 6, 8, 9, 12, 13)

_CACHE = {}


def _split_multi_waits(nc):
    n_split = 0
    for fn in nc.m.functions:
        for bb in fn.blocks:
            new_insts = []
            for inst in bb.instructions:
                si = inst.sync_info
                if si is not None and si.on_wait and len(si.on_wait) > 1:
                    waits = list(si.on_wait)
                    for w in waits[:-1]:
                        nop = mybir.InstNoOp(
                            name=f"{inst.name}-wsplit-{n_split}",
                            ins=[],
                            outs=[],
                            engine=inst.engine,
                            sync_info=mybir.SyncInfo(on_wait=[w], on_update=[]),
                        )
                        n_split += 1
                        new_insts.append(nop)
                    inst.sync_info = mybir.SyncInfo(
                        on_wait=[waits[-1]], on_update=list(si.on_update)
                    )
                new_insts.append(inst)
            bb.instructions = new_insts
    return n_split


def _build_kernel(split_waits=True):
    nc = bass.Bass(trn_type="TRN2")
    xt = nc.dram_tensor("xt", [BIN, 128, 2, T], BF16, kind="ExternalInput")
    vt_w = nc.dram_tensor("vt_w", [128, BIN, 2, RANK], BF16, kind="ExternalInput")
    u_w = nc.dram_tensor("u_w", [128, BOUT, BSOUT], BF16, kind="ExternalInput")
    s_w = nc.dram_tensor("s_w", [128, BOUT * BIN], F32, kind="ExternalInput")
    bias_w = nc.dram_tensor("bias_w", [128, BOUT * 2], F32, kind="ExternalInput")
    ot = nc.dram_tensor("ot", [BOUT, 128, 2, T], F32, kind="ExternalOutput")
    mult = mybir.AluOpType.mult
    add = mybir.AluOpType.add

    with TileContext(nc) as tc:
        with (
            tc.tile_pool(name="wpool", bufs=1) as wpool,
            tc.tile_pool(name="xpool", bufs=4) as xpool,
            tc.tile_pool(name="ypool", bufs=BIN + 2) as ypool,
            tc.tile_pool(name="dpool", bufs=8) as dpool,
            tc.tile_pool(name="zpool", bufs=4) as zpool,
            tc.tile_pool(name="opool", bufs=4) as opool,
            tc.tile_pool(name="ypsum", bufs=2, space="PSUM") as ypsum,
            tc.tile_pool(name="zpsum", bufs=1, space="PSUM") as zpsum,
            tc.tile_pool(name="opsum", bufs=2, space="PSUM") as opsum,
        ):
            vt_t = wpool.tile([128, BIN, 2, RANK], BF16)
            nc.sync.dma_start(out=vt_t, in_=vt_w[:, :, :, :])
            u_t = wpool.tile([128, BOUT, BSOUT], BF16)
            nc.sync.dma_start(out=u_t, in_=u_w[:, :, :])
            s_t = wpool.tile([128, BOUT * BIN], F32)
            nc.sync.dma_start(out=s_t, in_=s_w[:, :])
            bias_t = wpool.tile([128, BOUT * 2], F32)
            nc.sync.dma_start(out=bias_t, in_=bias_w[:, :])
            ident = wpool.tile([128, 128], BF16)
            make_identity(nc, ident)

            # ---- stage 1, both blocks: y_j (128, T) bf16 ----
            y_sb = []
            for j in range(BIN):
                x_t = xpool.tile([128, 2, T], BF16, tag="xt")
                nc.sync.dma_start(out=x_t, in_=xt[j, :, :, :])
                y = ypool.tile([128, T], BF16, tag="y")
                for blk in range(NBLK):
                    tok = slice(blk * NTOK, (blk + 1) * NTOK)
                    y_ps = ypsum.tile([128, NTOK], F32)
                    for k in range(2):
                        nc.tensor.matmul(
                            y_ps,
                            vt_t[:, j, k, :],
                            x_t[:, k, tok],
                            start=(k == 0),
                            stop=(k == 1),
                        )
                    nc.scalar.copy(y[:, tok], y_ps)
                y_sb.append(y)

            # ---- stage 2 + 3 per o, interleaving PE and DVE chains ----
            pe_os = [o for o in range(BOUT) if o in PE_OS]
            dve_os = [o for o in range(BOUT) if o not in PE_OS]
            # 2:1 PE:DVE interleave (~28us PE vs ~23us DVE per group)
            # to shrink PE idle gaps below the HAM re-throttle window.
            order = []
            pi, di = 0, 0
            while pi < len(pe_os) or di < len(dve_os):
                for _ in range(2):
                    if pi < len(pe_os):
                        order.append(("pe", pe_os[pi]))
                        pi += 1
                if di < len(dve_os):
                    order.append(("dve", dve_os[di]))
                    di += 1

            for kind, o in order:
                z = zpool.tile([128, T], BF16, tag="z")
                if kind == "pe":
                    zps = zpsum.tile([128, 2, NTOK], F32)
                    for j in range(BIN):
                        dg = dpool.tile([128, 128], BF16, tag="diag")
                        nc.vector.tensor_scalar(
                            dg, ident, s_t[:, o * BIN + j : o * BIN + j + 1],
                            None, mult,
                        )
                        for blk in range(NBLK):
                            nc.tensor.matmul(
                                zps[:, blk, :],
                                dg,
                                y_sb[j][:, blk * NTOK : (blk + 1) * NTOK],
                                start=(j == 0),
                                stop=(j == BIN - 1),
                            )
                    nc.scalar.copy(z, zps)
                else:
                    nc.scalar.mul(z, y_sb[0], s_t[:, o * BIN : o * BIN + 1])
                    for j in range(1, BIN):
                        nc.vector.scalar_tensor_tensor(
                            z, y_sb[j], s_t[:, o * BIN + j : o * BIN + j + 1],
                            z, mult, add,
                        )
                for blk in range(NBLK):
                    tok = slice(blk * NTOK, (blk + 1) * NTOK)
                    o_sb = opool.tile([128, 2, NTOK], F32, tag="o")
                    for h in range(2):
                        o_ps = opsum.tile([128, NTOK], F32)
                        nc.tensor.matmul(
                            o_ps,
                            u_t[:, o, 128 * h : 128 * (h + 1)],
                            z[:, tok],
                            start=True,
                            stop=True,
                        )
                        nc.scalar.activation(
                            o_sb[:, h, :],
                            o_ps,
                            mybir.ActivationFunctionType.Identity,
                            bias=bias_t[:, 2 * o + h : 2 * o + h + 1],
                            scale=1.0,
                        )
                    nc.sync.dma_start(out=ot[o, :, :, tok], in_=o_sb)

    if split_waits:
        _split_multi_waits(nc)
    return nc


def kernel(x, S, U, Vt, bias):
    x = np.asarray(x, dtype=np.float32)
    S = np.asarray(S, dtype=np.float32)
    U = np.asarray(U, dtype=np.float32)
    Vt = np.asarray(Vt, dtype=np.float32)
    bias = np.asarray(bias, dtype=np.float32)

    bf = ml_dtypes.bfloat16
    vt_w = np.ascontiguousarray(
        Vt.reshape(BIN, 2, 128, RANK).transpose(2, 0, 1, 3).astype(bf)
    )
    u_w = np.ascontiguousarray(U.transpose(1, 0, 2).astype(bf))
    s_w = np.ascontiguousarray(S.transpose(2, 0, 1).reshape(128, BOUT * BIN))
    bias_w = np.ascontiguousarray(
        bias.reshape(BOUT, 2, 128).transpose(2, 0, 1).reshape(128, BOUT * 2)
    )

    if "nc" not in _CACHE:
        _CACHE["nc"] = _build_kernel()
    nc = _CACHE["nc"]

    in_maps = []
    for b in range(B):
        xt = np.ascontiguousarray(
            x[b].T.reshape(BIN, 2, 128, T).transpose(0, 2, 1, 3).astype(bf)
        )
        in_maps.append(
            {"xt": xt, "vt_w": vt_w, "u_w": u_w, "s_w": s_w, "bias_w": bias_w}
        )

    res = run_bass_kernel_spmd(nc, in_maps, core_ids=list(range(B)))

    out = np.empty((B, T, D), dtype=np.float32)
    for b in range(B):
        o = res.results[b]["ot"]
        out[b] = o.transpose(3, 0, 2, 1).reshape(T, D)
    return out

